# revision 41
# baseline (speedup 1.0000x reference)
"""3-layer GCN (DeepGCN, PyG GCNConv semantics) on 8 Trainium2 NeuronCores.

v2 strategy (dst-sharded):
  - Nodes partitioned contiguously across 8 cores (6250 each, padded to
    6272 = 49*128). Edges partitioned by destination core, bucketed by
    (dst 128-tile, src table half), sorted by src, padded to a uniform
    chunk grid. Layer-1 grid additionally contains the self-loops.
  - Tables (the dinv[src]-scaled per-layer features) are stored fp8-e4m3,
    256 wide (layer 3 zero-padded), so every gather row is 256 B.
  - Layer 1: every core computes the FULL dense transform z1 = dinv*(x@W0)
    for all 8 cores' rows locally (dense is cheap) -> no AllGather.
    Layers 2/3: dense of own rows only + AllGather of the fp8 table,
    split into A/B halves so the collective pipelines with compute.
  - Aggregation per layer runs in two phases: all tiles' A-half chunks
    (accumulated to SBUF), then all B-half chunks (+A acc, +self, copyout).
    The layer-(l+1) dense is emitted per tile right after copyout, so the
    next AllGather starts while the B phase is still running.
  - Scatter-adds are one-hot-mask matmuls in PSUM; gathers are dma_gather
    calls round-robined over the SWDGE queues so descriptor generation
    overlaps across Q7 core pairs.
"""

import math

import numpy as np
import ml_dtypes

P = 128
NCORES = 8

BF16 = ml_dtypes.bfloat16

NQ = 4  # SWDGE queues


def _edge_grid(src, dst, npc, T, R, SA):
    """Bucket edges by (dst core, dst tile, src half); return per-core
    gather indices (int16, dma_gather layout) and dst-lane arrays."""
    SB = R - SA
    core = dst // npc
    dl = dst - core * npc
    t = dl // P
    dloc = dl % P
    c_src = src // npc
    l_src = src % npc
    half = (l_src >= SA).astype(np.int64)
    sloc = np.where(half == 1, c_src * SB + (l_src - SA),
                    c_src * SA + l_src).astype(np.int64)

    key = (core * T + t) * 2 + half
    nkeys = NCORES * T * 2
    cnt = np.bincount(key, minlength=nkeys)
    CA = max(int(math.ceil(cnt.reshape(-1, 2)[:, 0].max() / P)), 1)
    CB = max(int(math.ceil(cnt.reshape(-1, 2)[:, 1].max() / P)), 1)
    CT = CA + CB
    # per-(tile, half) worst-case row counts over cores, rounded to 16
    # (the dma_gather num_idxs granularity) -- used to trim pad lanes
    c3 = cnt.reshape(NCORES, T, 2)
    nidxA = [int(-16 * (-(c3[:, t, 0].max()) // 16)) for t in range(T)]
    nidxB = [int(-16 * (-(c3[:, t, 1].max()) // 16)) for t in range(T)]

    order = np.lexsort((sloc, key))
    key_s = key[order]
    starts = np.zeros(nkeys + 1, dtype=np.int64)
    starts[1:] = np.cumsum(cnt)
    rank = np.arange(key.shape[0], dtype=np.int64) - starts[key_s]

    t_s = (key_s // 2) % T
    half_s = key_s % 2
    chunk = t_s * CT + half_s * CA + rank // P
    lane = rank % P
    slot = chunk * P + lane
    core_s = key_s // (T * 2)

    nslots = T * CT * P
    gidx = np.zeros((NCORES, nslots), dtype=np.int16)
    gdloc = np.full((NCORES, nslots), 255, dtype=np.uint8)
    gidx[core_s, slot] = sloc[order].astype(np.int16)
    gdloc[core_s, slot] = dloc[order].astype(np.uint8)

    g3 = gidx.reshape(NCORES, T, CT, P)
    d3 = gdloc.reshape(NCORES, T, CT, P)

    def wrap16(flat):
        # dma_gather index layout: index i at [i % 16, i // 16], replicated
        # across the 8 Q7 cores (16-partition groups)
        m = flat.reshape(-1, 16).T.copy()
        return np.tile(m, (8, 1))

    idxA = np.stack([wrap16(g3[c, :, :CA, :].reshape(-1)) for c in range(NCORES)])
    idxB = np.stack([wrap16(g3[c, :, CA:, :].reshape(-1)) for c in range(NCORES)])
    dlocb = np.ascontiguousarray(
        d3.reshape(NCORES, T * CT, P).transpose(0, 2, 1))
    return dict(CA=CA, CB=CB, idxA=idxA, idxB=idxB, dlocb=dlocb,
                nidxA=nidxA, nidxB=nidxB)


def _build_nc(D, NCLS, T, R, CA1, CB1, CA, CB, nidx1, nidx23):
    import concourse.bacc as bacc
    import concourse.mybir as mybir
    import concourse.tile as tile
    from concourse.masks import make_identity

    dt = mybir.dt
    F8 = dt.float8e4
    CT1 = CA1 + CB1
    CT = CA + CB
    CAm = max(CA1, CA)
    CBm = max(CB1, CB)
    CTm = max(CT1, CT)
    TA = T // 2
    SA = TA * P
    SB = R - SA
    HA = NCORES * SA
    HB = NCORES * SB
    W3 = 256                      # padded width of layer-3 table (fp8)
    LA1 = T * CA1 * P
    LB1 = T * CB1 * P
    LA = T * CA * P
    LB = T * CB * P
    LAm = max(LA1, LA)
    LBm = max(LB1, LB)

    nc = bacc.Bacc("TRN2", target_bir_lowering=False, debug=False,
                   num_devices=NCORES, num_swdge_queues=NQ)
    xTb = nc.dram_tensor("xTb", [2 * P, NCORES * R], F8,
                         kind="ExternalInput")
    W0b = nc.dram_tensor("W0b", [D, D], F8, kind="ExternalInput")
    W1b = nc.dram_tensor("W1b", [D, D], dt.bfloat16, kind="ExternalInput")
    W2b = nc.dram_tensor("W2b", [D, W3], dt.bfloat16, kind="ExternalInput")
    dinvfb = nc.dram_tensor("dinvfb", [P, NCORES * T], dt.float32,
                            kind="ExternalInput")
    dinvob = nc.dram_tensor("dinvob", [P, T], dt.float32, kind="ExternalInput")
    dloc1b = nc.dram_tensor("dloc1b", [P, T * CT1], dt.uint8,
                            kind="ExternalInput")
    dlocb = nc.dram_tensor("dlocb", [P, T * CT], dt.uint8,
                           kind="ExternalInput")
    iotarb = nc.dram_tensor("iotarb", [P, CTm * P], dt.uint8,
                            kind="ExternalInput")
    idxA1b = nc.dram_tensor("idxA1b", [P, LA1 // 16], dt.int16,
                            kind="ExternalInput")
    idxB1b = nc.dram_tensor("idxB1b", [P, LB1 // 16], dt.int16,
                            kind="ExternalInput")
    idxAb = nc.dram_tensor("idxAb", [P, LA // 16], dt.int16,
                           kind="ExternalInput")
    idxBb = nc.dram_tensor("idxBb", [P, LB // 16], dt.int16,
                           kind="ExternalInput")
    outb = nc.dram_tensor("out", [P, T * NCLS], dt.float32,
                          kind="ExternalOutput")

    AF = mybir.ActivationFunctionType
    ALU = mybir.AluOpType
    rg = [list(range(NCORES))]

    with tile.TileContext(nc) as tc:
        with (
            tc.tile_pool(name="const", bufs=1) as cpool,
            tc.tile_pool(name="work", bufs=3) as wpool,
            tc.tile_pool(name="xa", bufs=2) as xapool,
            tc.tile_pool(name="mpool", bufs=4) as mpool,
            tc.tile_pool(name="gpool", bufs=8) as gpool,
            tc.tile_pool(name="pa", bufs=3, space="PSUM") as pa,
            tc.tile_pool(name="pd", bufs=2, space="PSUM") as pd,
            tc.tile_pool(name="pt", bufs=2, space="PSUM") as pt,
            tc.tile_pool(name="dram", bufs=1, space="DRAM") as dram,
        ):
            # ---- resident constants ----
            dinvf_sb = cpool.tile([P, NCORES * T], dt.float32, tag="dinvf")
            nc.sync.dma_start(out=dinvf_sb[:], in_=dinvfb[:])
            dinvo_sb = cpool.tile([P, T], dt.float32, tag="dinvo")
            nc.sync.dma_start(out=dinvo_sb[:], in_=dinvob[:])
            dloc1_sb = cpool.tile([P, T * CT1], dt.uint8, tag="dloc1")
            nc.sync.dma_start(out=dloc1_sb[:], in_=dloc1b[:])
            dloc_sb = cpool.tile([P, T * CT], dt.uint8, tag="dloc")
            nc.sync.dma_start(out=dloc_sb[:], in_=dlocb[:])
            iotar_sb = cpool.tile([P, CTm * P], dt.uint8, tag="iotar")
            nc.sync.dma_start(out=iotar_sb[:], in_=iotarb[:])
            # idx buffers: hold layer-1 grids first, then reloaded with the
            # layer-2/3 grids after the layer-1 gathers have consumed them.
            idxA_sb = cpool.tile([P, LAm // 16], dt.int16, tag="idxA")
            nc.sync.dma_start(out=idxA_sb[:, :LA1 // 16], in_=idxA1b[:])
            idxB_sb = cpool.tile([P, LBm // 16], dt.int16, tag="idxB")
            nc.sync.dma_start(out=idxB_sb[:, :LB1 // 16], in_=idxB1b[:])
            ident_sb = cpool.tile([P, P], dt.bfloat16, tag="ident")
            make_identity(nc, ident_sb[:])
            W0_sb = cpool.tile([P, 2 * D], F8, tag="W0")
            W1_sb = cpool.tile([P, 2 * D], dt.bfloat16, tag="W1")
            W2_sb = cpool.tile([P, 2 * W3], dt.bfloat16, tag="W2")
            for h in (0, 1):
                nc.sync.dma_start(out=W0_sb[:, h * D:(h + 1) * D],
                                  in_=W0b[h * P:(h + 1) * P, :])
                nc.sync.dma_start(out=W1_sb[:, h * D:(h + 1) * D],
                                  in_=W1b[h * P:(h + 1) * P, :])
                nc.sync.dma_start(out=W2_sb[:, h * W3:(h + 1) * W3],
                                  in_=W2b[h * P:(h + 1) * P, :])

            z_acc = cpool.tile([P, T * D], F8, tag="zacc")        # layer-2 self
            z3_acc = cpool.tile([P, T * W3], F8, tag="z3acc")     # layer-3 self
            h1_acc = cpool.tile([P, T * D], dt.bfloat16, tag="h1")
            accA = cpool.tile([P, T * D], dt.bfloat16, tag="accA")
            out_acc = cpool.tile([P, T * NCLS], dt.float32, tag="oacc")

            # ---- DRAM scratch ----
            t1a = dram.tile([HA, D], F8, tag="t1a")
            t1b = dram.tile([HB, D], F8, tag="t1b")
            bounce2 = dram.tile([R, D], F8, tag="b2")
            t2a = dram.tile([HA, D], F8, tag="t2a", addr_space="Shared")
            t2b = dram.tile([HB, D], F8, tag="t2b", addr_space="Shared")
            bounce3 = dram.tile([R, W3], F8, tag="b3")
            t3a = dram.tile([HA, W3], F8, tag="t3a", addr_space="Shared")
            t3b = dram.tile([HB, W3], F8, tag="t3b", addr_space="Shared")

            # ------------- layer-1 full dense: z1 for ALL 8*T tiles -------------
            DR = mybir.MatmulPerfMode.DoubleRow

            def dense1_half(half):
                nt = TA if half == 0 else T - TA
                t0 = 0 if half == 0 else TA
                for c in range(NCORES):
                    # fp8 xT block, h-major: xa[p, j*nt*P + t*P + m] = xT[j*128+p, ...]
                    xa = xapool.tile([P, (T - TA) * 2 * P], F8, tag="xa")
                    for h in (0, 1):
                        nc.sync.dma_start(
                            out=xa[:, h * nt * P:(h + 1) * nt * P],
                            in_=xTb[h * P:(h + 1) * P,
                                    c * R + t0 * P:c * R + (t0 + nt) * P])
                    xa4 = xa[:, :2 * nt * P].rearrange(
                        "p (j t m) -> p t j m", j=2, m=P)
                    # staging for the table rows (z_acc / z3_acc are free here)
                    stage = z_acc if c % 2 == 0 else z3_acc
                    for ti in range(nt):
                        t = t0 + ti
                        psd = pd.tile([P, D], dt.float32, tag="psd")
                        nc.tensor.matmul(
                            psd[:],
                            lhsT=xa4[:, ti],
                            rhs=W0_sb[:].rearrange("p (j n) -> p j n", j=2),
                            perf_mode=DR, start=True, stop=True)
                        gt = c * T + t
                        if c % 2 == 0:
                            nc.scalar.activation(stage[:, ti * D:(ti + 1) * D],
                                                 psd[:], AF.Copy,
                                                 scale=dinvf_sb[:, gt:gt + 1])
                        else:
                            nc.vector.tensor_tensor(
                                out=stage[:, ti * D:(ti + 1) * D], in0=psd[:],
                                in1=dinvf_sb[:, gt:gt + 1].to_broadcast([P, D]),
                                op=ALU.mult)
                    dst = (t1a[c * SA:(c + 1) * SA, :] if half == 0
                           else t1b[c * SB:(c + 1) * SB, :])
                    # on the scalar queue so it doesn't block xa prefetches
                    nc.scalar.dma_start(
                        out=dst.rearrange("(t p) d -> p t d", p=P),
                        in_=stage[:, :nt * D].rearrange("p (t d) -> p t d", d=D))

            # ------------- dense for layers 2/3 (own tiles, via PE transpose) ----
            def dense_next(layer, t, hsrc):
                # hsrc: SBUF AP [P, D] holding h_layer tile t (bf16)
                W_sb = W1_sb if layer == 2 else W2_sb
                width = D if layer == 2 else W3
                hts = []
                for h in (0, 1):
                    pst = pt.tile([P, P], dt.bfloat16, tag="pst")
                    nc.tensor.transpose(pst[:], hsrc[:, h * P:(h + 1) * P],
                                        ident_sb[:])
                    hT = wpool.tile([P, P], dt.bfloat16, tag="hT")
                    nc.vector.tensor_copy(hT[:], pst[:])
                    hts.append(hT)
                psd = pd.tile([P, W3], dt.float32, tag="psd")
                for h in (0, 1):
                    nc.tensor.matmul(psd[:, :width], lhsT=hts[h][:],
                                     rhs=W_sb[:, h * width:(h + 1) * width],
                                     start=(h == 0), stop=(h == 1))
                if layer == 2:
                    zs = z_acc[:, t * D:(t + 1) * D]
                    bounce = bounce2
                    rows = slice(t * P, (t + 1) * P)
                else:
                    zs = z3_acc[:, t * W3:(t + 1) * W3]
                    bounce = bounce3
                    rows = slice(t * P, (t + 1) * P)
                nc.scalar.activation(zs, psd[:, :width], AF.Copy,
                                     scale=dinvo_sb[:, t:t + 1])
                nc.sync.dma_start(out=bounce[rows, :], in_=zs)

            # ------------- aggregation phases -------------
            def mask_for(dloc_t, ci, cn):
                # one-hot fp8 masks for chunks [ci, ci+cn) of this tile
                M = mpool.tile([P, max(CAm, CBm) * P], F8, tag="M")
                nc.vector.tensor_tensor(
                    out=M[:, :cn * P].rearrange("p (c o) -> p c o", o=P),
                    in0=iotar_sb[:, :cn * P].rearrange("p (c o) -> p c o", o=P),
                    in1=dloc_t[:, ci:ci + cn]
                        .rearrange("p (c o) -> p c o", o=1)
                        .to_broadcast([P, cn, P]),
                    op=ALU.is_equal)
                return M

            def chunk_mms(ps, M, g, cn):
                # pairs of chunks via fp8 DoubleRow, odd tail as a plain MM
                pairs = cn // 2
                for c in range(pairs):
                    nc.tensor.matmul(
                        ps[:],
                        lhsT=M[:, 2 * c * P:(2 * c + 2) * P]
                            .rearrange("p (j m) -> p j m", j=2),
                        rhs=g[:, 2 * c:2 * c + 2, :],
                        perf_mode=DR,
                        start=(c == 0), stop=(c == pairs - 1 and cn % 2 == 0))
                if cn % 2:
                    nc.tensor.matmul(ps[:],
                                     lhsT=M[:, (cn - 1) * P:cn * P],
                                     rhs=g[:, cn - 1, :],
                                     start=(cn == 1), stop=True)

            qctr = [0]

            def gathers_for(g, tsrc, idx_sb, t, cn, nidx):
                # split into <=6-chunk calls so each engine's packet stays
                # under the 64-descriptor packet ceiling (single_packet mode);
                # num_idxs trimmed to the worst-core row count for this bucket
                col0 = t * cn * 8
                n1 = (cn + 1) // 2
                for (c0, k) in ((0, n1), (n1, cn - n1)):
                    if k == 0:
                        continue
                    ni = min(max(nidx - c0 * P, 0), k * P)
                    if ni == 0:
                        continue
                    kk = (ni + P - 1) // P     # chunks actually written
                    nc.gpsimd.dma_gather(
                        g[:, c0:c0 + kk, :], tsrc[:],
                        idx_sb[:, col0 + c0 * 8:col0 + c0 * 8 + (ni + 15) // 16],
                        ni, ni, D, single_packet=True,
                        queue_num=qctr[0] % NQ)
                    qctr[0] += 1

            def phaseA(ta, dloc_sbuf, CAl, CTl, idx_sb, nidx):
                for t in range(T):
                    g = gpool.tile([P, CAm, D], F8, tag="gA")
                    gathers_for(g, ta, idx_sb, t, CAl, nidx[t])
                    M = mask_for(dloc_sbuf[:, t * CTl:(t + 1) * CTl], 0, CAl)
                    ps = pa.tile([P, D], dt.float32, tag="ps")
                    chunk_mms(ps, M, g, CAl)
                    nc.vector.tensor_copy(accA[:, t * D:(t + 1) * D], ps[:])

            def phaseB(tb, dloc_sbuf, CBl, CAl, CTl, idx_sb, nidx, copyout,
                       inject=None):
                for t in range(T):
                    if inject is not None and t == TA + 6:
                        inject()
                    g = gpool.tile([P, CBm, D], F8, tag="gB")
                    gathers_for(g, tb, idx_sb, t, CBl, nidx[t])
                    M = mask_for(dloc_sbuf[:, t * CTl:(t + 1) * CTl], CAl, CBl)
                    ps = pa.tile([P, D], dt.float32, tag="ps")
                    chunk_mms(ps, M, g, CBl)
                    nc.vector.tensor_tensor(
                        out=ps[:], in0=ps[:],
                        in1=accA[:, t * D:(t + 1) * D], op=ALU.add)
                    copyout(t, ps)

            def copyout1(t, ps):
                # self-loop already in the layer-1 edge grid
                hs = h1_acc[:, t * D:(t + 1) * D]
                nc.scalar.activation(hs, ps[:, :D], AF.Relu,
                                     scale=dinvo_sb[:, t:t + 1])
                dense_next(2, t, hs)

            def copyout2(t, ps):
                nc.vector.tensor_tensor(
                    out=ps[:, :D], in0=ps[:, :D],
                    in1=z_acc[:, t * D:(t + 1) * D], op=ALU.add)
                tmp = wpool.tile([P, D], dt.bfloat16, tag="tmp2")
                nc.scalar.activation(tmp[:], ps[:, :D], AF.Relu,
                                     scale=dinvo_sb[:, t:t + 1])
                h2 = wpool.tile([P, D], dt.bfloat16, tag="h2t")
                nc.vector.tensor_tensor(
                    out=h2[:], in0=tmp[:],
                    in1=h1_acc[:, t * D:(t + 1) * D], op=ALU.add)
                dense_next(3, t, h2)

            def copyout3(t, ps):
                nc.vector.tensor_tensor(
                    out=ps[:, :NCLS], in0=ps[:, :NCLS],
                    in1=z3_acc[:, t * W3:t * W3 + NCLS], op=ALU.add)
                u = wpool.tile([P, NCLS], dt.float32, tag="u")
                nc.scalar.activation(u[:], ps[:, :NCLS], AF.Copy,
                                     scale=dinvo_sb[:, t:t + 1])
                rmax = wpool.tile([P, 1], dt.float32, tag="rmax")
                nc.vector.reduce_max(rmax[:], u[:], axis=mybir.AxisListType.X)
                su = wpool.tile([P, NCLS], dt.float32, tag="su")
                nc.vector.tensor_tensor(
                    out=su[:], in0=u[:],
                    in1=rmax[:].to_broadcast([P, NCLS]), op=ALU.subtract)
                ex = wpool.tile([P, NCLS], dt.float32, tag="ex")
                nc.scalar.activation(ex[:], su[:], AF.Exp)
                ssum = wpool.tile([P, 1], dt.float32, tag="ssum")
                nc.vector.reduce_sum(ssum[:], ex[:], axis=mybir.AxisListType.X)
                lse = wpool.tile([P, 1], dt.float32, tag="lse")
                nc.scalar.activation(lse[:], ssum[:], AF.Ln)
                nc.vector.tensor_tensor(
                    out=out_acc[:, t * NCLS:(t + 1) * NCLS], in0=su[:],
                    in1=lse[:].to_broadcast([P, NCLS]), op=ALU.subtract)

            def ag_half(bounce, dst_t, half):
                if half == 0:
                    nc.gpsimd.collective_compute(
                        "AllGather", ALU.bypass,
                        ins=[bounce[0:SA, :]], outs=[dst_t.opt()],
                        replica_groups=rg,
                    )
                else:
                    nc.gpsimd.collective_compute(
                        "AllGather", ALU.bypass,
                        ins=[bounce[SA:R, :]], outs=[dst_t.opt()],
                        replica_groups=rg,
                    )

            # ================= program =================
            nc.vector.memset(out_acc[:], 0.0)
            # zero the gather buffers once: trailing -1 pad indices leave
            # regions unwritten, and uninitialized SBUF could decode as fp8
            # NaN which would poison the masked matmuls (NaN * 0 = NaN)
            for _ in range(8):
                ga0 = gpool.tile([P, CAm, D], F8, tag="gA")
                nc.vector.memset(ga0[:], 0.0)
                gb0 = gpool.tile([P, CBm, D], F8, tag="gB")
                nc.vector.memset(gb0[:], 0.0)
            # layer-1 dense: table A half first so A gathers can start early
            dense1_half(0)
            dense1_half(1)
            # ---- layer 1 aggregation (self-loops included in grid) ----
            nA1, nB1 = nidx1
            nA, nB = nidx23
            phaseA(t1a, dloc1_sb, CA1, CT1, idxA_sb, nA1)
            # AG2a is injected mid-phase so it starts as soon as the A-half
            # of dense2 (emitted per tile by copyout1) has landed in bounce2
            phaseB(t1b, dloc1_sb, CB1, CA1, CT1, idxB_sb, nB1, copyout1,
                   inject=lambda: ag_half(bounce2, t2a, 0))
            # reload idx buffers with the layer-2/3 grid
            nc.sync.dma_start(out=idxA_sb[:, :LA // 16], in_=idxAb[:])
            nc.sync.dma_start(out=idxB_sb[:, :LB // 16], in_=idxBb[:])
            ag_half(bounce2, t2b, 1)
            # ---- layer 2 aggregation ----
            phaseA(t2a, dloc_sb, CA, CT, idxA_sb, nA)
            phaseB(t2b, dloc_sb, CB, CA, CT, idxB_sb, nB, copyout2,
                   inject=lambda: ag_half(bounce3, t3a, 0))
            ag_half(bounce3, t3b, 1)
            # ---- layer 3 aggregation ----
            phaseA(t3a, dloc_sb, CA, CT, idxA_sb, nA)
            phaseB(t3b, dloc_sb, CB, CA, CT, idxB_sb, nB, copyout3)

            nc.sync.dma_start(out=outb[:], in_=out_acc[:])

    nc.compile()
    return nc


def kernel(**inputs):
    x = np.asarray(inputs["x"], dtype=np.float32)
    edge_index = np.asarray(inputs["edge_index"]).astype(np.int64)
    W0 = np.asarray(inputs["W0"], dtype=np.float32)
    b0 = np.asarray(inputs["b0"], dtype=np.float32)
    W1 = np.asarray(inputs["W1"], dtype=np.float32)
    b1 = np.asarray(inputs["b1"], dtype=np.float32)
    W2 = np.asarray(inputs["W2"], dtype=np.float32)
    b2 = np.asarray(inputs["b2"], dtype=np.float32)

    n, D = x.shape
    NCLS = W2.shape[1]
    npc = n // NCORES
    T = math.ceil(npc / P)
    R = T * P
    TA = T // 2
    SA = TA * P
    W3 = 256

    assert not (np.any(b0) or np.any(b1) or np.any(b2)), \
        "nonzero biases not implemented"

    deg = np.bincount(edge_index[1], minlength=n).astype(np.float64) + 1.0
    dinv = (1.0 / np.sqrt(deg)).astype(np.float32)

    src = edge_index[0].astype(np.int64)
    dst = edge_index[1].astype(np.int64)
    loops = np.arange(n, dtype=np.int64)
    g1 = _edge_grid(np.concatenate([src, loops]),
                    np.concatenate([dst, loops]), npc, T, R, SA)
    g23 = _edge_grid(src, dst, npc, T, R, SA)
    CA1, CB1 = g1["CA"], g1["CB"]
    CA, CB = g23["CA"], g23["CB"]
    CTm = max(CA1 + CB1, CA + CB)
    iotar = np.tile(np.arange(P, dtype=np.uint8), (P, CTm))

    from concourse.bass_utils import run_bass_kernel_spmd

    nc = _build_nc(D, NCLS, T, R, CA1, CB1, CA, CB,
                   (g1["nidxA"], g1["nidxB"]), (g23["nidxA"], g23["nidxB"]))

    W2p = np.zeros((D, W3), dtype=np.float32)
    W2p[:, :NCLS] = W2

    F8NP = ml_dtypes.float8_e4m3

    # full transposed feature table [256, NCORES*R] (shared by all cores)
    xf = np.zeros((NCORES * R, D), dtype=np.float32)
    for c in range(NCORES):
        xf[c * R:c * R + npc] = x[c * npc:(c + 1) * npc]
    xT = np.ascontiguousarray(xf.T).astype(F8NP)

    dvf = np.zeros((NCORES * R,), dtype=np.float32)
    for c in range(NCORES):
        dvf[c * R:c * R + npc] = dinv[c * npc:(c + 1) * npc]
    # W0 is stored fp8 prescaled by 16 (keeps small weights out of the fp8
    # subnormal range); the 1/16 is folded into the dense-1 dinv scale.
    dvfb = np.ascontiguousarray(dvf.reshape(NCORES * T, P).T) / 16.0

    in_maps = []
    for c in range(NCORES):
        dv = np.zeros((T * P,), dtype=np.float32)
        dv[:npc] = dinv[c * npc:(c + 1) * npc]
        dvb = np.ascontiguousarray(dv.reshape(T, P).T)
        in_maps.append({
            "xTb": xT,
            "W0b": (W0 * 16.0).astype(F8NP),
            "W1b": W1.astype(BF16),
            "W2b": W2p.astype(BF16),
            "dinvfb": dvfb,
            "dinvob": dvb,
            "dloc1b": g1["dlocb"][c],
            "dlocb": g23["dlocb"][c],
            "iotarb": iotar,
            "idxA1b": g1["idxA"][c],
            "idxB1b": g1["idxB"][c],
            "idxAb": g23["idxA"][c],
            "idxBb": g23["idxB"][c],
        })

    res = run_bass_kernel_spmd(nc, in_maps, core_ids=list(range(NCORES)))
    global _LAST_RESULTS
    _LAST_RESULTS = res

    out = np.empty((n, NCLS), dtype=np.float32)
    for c in range(NCORES):
        oc = res.results[c]["out"]                            # [128, T*NCLS]
        oc = oc.reshape(P, T, NCLS).transpose(1, 0, 2).reshape(R, NCLS)
        out[c * npc:(c + 1) * npc] = oc[:npc]
    return out


if __name__ == "__main__":
    pass
